# revision 8
# baseline (speedup 1.0000x reference)
"""Trainium2 Bass kernel for nn_Net4 (hypernetwork RNN scan), v2.

Model (per step t, per batch row b):
  h1 = sigmoid(m @ A1 + pre1[t])          A1 = W_enc_w[:64]
  h2 = sigmoid(m @ B1 + pre2[t])          B1 = b_enc_w[:64]
  Wm = (h1 @ W_dec_w + W_dec_b).reshape(64,64)
  bm = h2 @ b_dec_w + b_dec_b
  m' = sigmoid(Wm @ m + bm)
  loss[t] = (logsumexp(m'@dec_w+dec_b) - (m'@dec_w+dec_b)[y]) / ln2

v2 changes vs v1:
  - everything in the scan is bf16 (v1 had fp32 wbT/bw65/tsb matmuls that
    cost 2x LDWEIGHTS+MATMUL at ~200ns each)
  - pre1/pre2 are pre-written into the g PSUM bank by the vector engine
    (interleaved preg layout), and the g matmuls accumulate on top with
    start=False; this removes the vector add from the critical path
  - single m_bf [64,2]: all T-chunk stationaries live on partitions 0-63
    so no duplicated m halves and only one sigmoid per step boundary
  - m_hist is a bf16 copy of m_bf done by the vector engine (off the
    critical path), feeding a bf16 bulk loss phase (v1 bulk was fp32 and
    took ~106us; bf16 takes ~15us)

Sharding: batch rows 2k,2k+1 -> core k; zero cross-core communication.
"""

import os
import sys
import numpy as np

sys.path.insert(0, "/opt/trn_rl_repo")

import concourse.bass as bass
import concourse.bacc as bacc
import concourse.mybir as mybir
import concourse.tile as tile
from concourse.bass_utils import run_bass_kernel_spmd

import ml_dtypes

BF16 = ml_dtypes.bfloat16

Cin, E, L, M, H, Cout = 256, 16, 64, 64, 64, 256
B, N = 16, 2048
D = M + L * E  # 1088
NCORES = 8
BL = B // NCORES  # 2 batch rows per core
NB = N * BL       # 4096 (t,b) pairs per core
TAU = N + L - 8   # e8 time length: tau in [0, 2104)
E8COLS = TAU * BL  # 4208

F32 = mybir.dt.float32
BF16_DT = mybir.dt.bfloat16
AF = mybir.ActivationFunctionType
FP8_DT = mybir.dt.float8e4
FP8 = ml_dtypes.float8_e4m3fn
INV_LN2 = float(1.0 / np.log(2.0))

_cache = {}


def _build_nc(unroll=16):
    nc = bacc.Bacc("TRN2", target_bir_lowering=False, debug=True)

    # ---- DRAM parameters (per-core inputs) ----
    def P(name, shape, dt):
        return nc.declare_dram_parameter(name, list(shape), dt, isOutput=False)

    e8_d = P("e8", (128, E8COLS), BF16_DT)
    wpre1_d = P("wpre1", (128, 8 * 128), BF16_DT)
    wpre2_d = P("wpre2", (128, 8 * 64), BF16_DT)
    bias1_d = P("bias1", (1, 128), BF16_DT)   # [W_enc_b | W_enc_b]
    bias2_d = P("bias2", (1, 64), BF16_DT)    # b_enc_b
    gse_d = P("gse", (64, 128), BF16_DT)      # [A1 | A1]
    gso_d = P("gso", (64, 128), BF16_DT)      # [B1 | B1]
    wstatT_d = P("wstatT", (64, 32 * 128), BF16_DT)
    wbT_d = P("wbT", (64, 64), BF16_DT)       # W_dec_b reshaped [j,i]
    bw65_d = P("bw65", (65, 64), BF16_DT)     # [b_dec_w ; b_dec_b]
    decstat_d = P("decstat", (65, 256), BF16_DT)  # [dec_w ; dec_b]
    gaug_d = P("gaug", (65, NB), BF16_DT)     # picked dec cols * inv_ln2
    ones65_d = P("ones65", (65, 1), BF16_DT)
    ones128_d = P("ones128", (128, 1), BF16_DT)
    out_d = nc.declare_dram_parameter("out", [1, NB], F32, isOutput=True)

    with tile.TileContext(nc) as tc:
        with (
            tc.tile_pool(name="persist", bufs=1) as pp,
            tc.tile_pool(name="psum", bufs=2, space="PSUM") as psp,
        ):
            e8 = pp.tile([128, E8COLS], BF16_DT)
            wpre1 = pp.tile([128, 8 * 128], BF16_DT)
            wpre2 = pp.tile([128, 8 * 64], BF16_DT)
            bias1 = pp.tile([1, 128], BF16_DT)
            bias2 = pp.tile([1, 64], BF16_DT)
            gse = pp.tile([64, 128], BF16_DT)
            gso = pp.tile([64, 128], BF16_DT)
            wstatT = pp.tile([64, 32 * 128], BF16_DT)
            wbT = pp.tile([64, 64], BF16_DT)
            bw65 = pp.tile([65, 64], BF16_DT)
            decstat = pp.tile([65, 256], BF16_DT)
            gaug = pp.tile([65, NB], BF16_DT)
            ones65 = pp.tile([65, 1], BF16_DT)
            ones128 = pp.tile([128, 1], BF16_DT)

            for sb, dr in [
                (e8, e8_d), (wpre1, wpre1_d), (wpre2, wpre2_d),
                (bias1, bias1_d), (bias2, bias2_d),
                (gse, gse_d), (gso, gso_d),
                (wstatT, wstatT_d), (wbT, wbT_d),
                (bw65, bw65_d), (decstat, decstat_d),
                (gaug, gaug_d), (ones65, ones65_d), (ones128, ones128_d),
            ]:
                nc.default_dma_engine.dma_start(sb[:], dr[:])

            preg = pp.tile([128, N, 4], F32)      # pre1 (dup) | pre2 per t
            m_hist = pp.tile([65, NB + 2 * BL], BF16_DT)  # row 64 == 1.0
            m_bf = pp.tile([64, BL], BF16_DT)
            h1d = pp.tile([128, BL], BF16_DT)
            h2t = pp.tile([65, BL], BF16_DT)      # row 64 == 1.0
            tsb = pp.tile([128, BL, 32], BF16_DT)
            onerow = pp.tile([1, 512], BF16_DT)
            zeros4 = pp.tile([64, 4], BF16_DT)

            nc.vector.memset(m_hist[0:64, 0:BL], 0.0)
            nc.vector.memset(m_hist[64:65, :], 1.0)
            nc.vector.memset(m_bf[:], 0.0)
            nc.vector.memset(h2t[64:65, :], 1.0)
            nc.vector.memset(onerow[:], 1.0)
            nc.vector.memset(zeros4[:], 0.0)

            # ---- precompute preg = [pre1(b0),pre1(b1),pre2(b0),pre2(b1)] ----
            for n in range(8):
                ps1 = psp.tile([128, 256, BL], F32, tag="big")
                for c in range(8):
                    nc.tensor.matmul(
                        ps1[:],
                        wpre1[:, c * 128:(c + 1) * 128],
                        e8[:, 16 * c + 512 * n: 16 * c + 512 * n + 512],
                        start=(c == 0), stop=False,
                    )
                nc.tensor.matmul(ps1[:], bias1[:], onerow[:],
                                 start=False, stop=True, skip_group_check=True)
                nc.vector.tensor_copy(preg[:, 256 * n:256 * (n + 1), 0:2],
                                      ps1[:])
                ps2 = psp.tile([64, 256, BL], F32, tag="big")
                for c in range(8):
                    nc.tensor.matmul(
                        ps2[:],
                        wpre2[:, c * 64:(c + 1) * 64],
                        e8[:, 16 * c + 512 * n: 16 * c + 512 * n + 512],
                        start=(c == 0), stop=False,
                    )
                nc.tensor.matmul(ps2[:], bias2[:], onerow[:],
                                 start=False, stop=True, skip_group_check=True)
                nc.vector.tensor_copy(preg[0:64, 256 * n:256 * (n + 1), 2:4],
                                      ps2[:])

            # warm up the two g PSUM banks so their has_written bits are set
            # (the in-loop g matmuls use start=False over a DVE pre-write)
            for _ in range(2):
                gw = psp.tile([128, 1, 4], F32, tag="g_ps")
                nc.tensor.matmul(gw[:, 0, :], gse[:], zeros4[:],
                                 start=True, stop=True)

            # ---- the scan ----
            with tc.For_i(0, N, unroll,
                          hint_engines=(mybir.EngineType.PE,)) as iv:
                for k in range(unroll):
                    t = iv + k
                    tcol = t * BL
                    g_ps = psp.tile([128, 1, 4], F32, tag="g_ps")
                    a_ps = psp.tile([64, BL], F32, tag="a_ps", bufs=1)
                    T_psA = psp.tile([128, 16, BL], F32, tag="T_psA", bufs=1)
                    T_psB = psp.tile([128, 16, BL], F32, tag="T_psB", bufs=1)

                    # pre1/pre2 pre-written into the g bank (vector, early)
                    nc.vector.tensor_copy(g_ps[:], preg[:, bass.ds(t, 1), :])

                    # g = [A1|A1]^T m (+pre1), [B1|B1]^T m (+pre2)
                    nc.tensor.matmul(g_ps[:, 0, 0:2], gse[:], m_bf[:],
                                     start=False, stop=True,
                                     skip_group_check=True)
                    nc.tensor.matmul(g_ps[:, 0, 2:4], gso[:], m_bf[:],
                                     start=False, stop=True,
                                     skip_group_check=True)
                    # T chunks: T[p, c, b]: p<64 -> T[h=p, i=c],
                    #                       p>=64 -> T[h=p-64, i=c+32]
                    for c in range(16):
                        nc.tensor.matmul(
                            T_psA[:, c, :],
                            wstatT[:, c * 128:(c + 1) * 128],
                            m_bf[:], start=True, stop=True)
                    for c in range(16, 32):
                        nc.tensor.matmul(
                            T_psB[:, c - 16, :],
                            wstatT[:, c * 128:(c + 1) * 128],
                            m_bf[:], start=True, stop=True)

                    # h = sigmoid(g) (g already includes pre via pre-write)
                    nc.scalar.activation(h1d[:], g_ps[:, 0, 0:2], AF.Sigmoid)
                    nc.scalar.activation(h2t[0:64, :], g_ps[0:64, 0, 2:4],
                                         AF.Sigmoid)

                    # T -> SBUF (bf16); A copy overlaps the B-chunk matmuls
                    nc.vector.tensor_copy(
                        tsb[:, :, 0:16].transpose([0, 2, 1]), T_psA[:])
                    nc.vector.tensor_copy(
                        tsb[:, :, 16:32].transpose([0, 2, 1]), T_psB[:])

                    # a = WbT@m + bw65@[h2;1] + sum_h h1*T
                    nc.tensor.matmul(a_ps[:], wbT[:], m_bf[:],
                                     start=True, stop=False)
                    nc.tensor.matmul(a_ps[:], bw65[:], h2t[:],
                                     start=False, stop=False,
                                     skip_group_check=True)
                    for b in range(BL):
                        nc.tensor.matmul(a_ps[0:32, b: b + 1],
                                         tsb[0:64, b, :], h1d[0:64, b: b + 1],
                                         start=False, stop=False,
                                         skip_group_check=True,
                                         tile_position=(0, 0))
                        last = b == BL - 1
                        nc.tensor.matmul(a_ps[32:64, b: b + 1],
                                         tsb[64:128, b, :],
                                         h1d[64:128, b: b + 1],
                                         start=False, stop=last,
                                         skip_group_check=True,
                                         tile_position=(64, 32))

                    # m' = sigmoid(a)
                    nc.scalar.activation(m_bf[:], a_ps[:], AF.Sigmoid)
                    nc.vector.tensor_copy(
                        m_hist[0:64, bass.ds(tcol + BL, BL)], m_bf[:])

            # ---- bulk loss ----
            loss = pp.tile([1, NB], F32)
            with tc.tile_pool(name="bulk", bufs=2) as bp:
                for tcn in range(8):
                    sl = slice(512 * tcn, 512 * (tcn + 1))
                    msl = slice(BL + 512 * tcn, BL + 512 * (tcn + 1))
                    se_ps = psp.tile([1, 512], F32, tag="seps", bufs=1)
                    for half in range(2):
                        lg_ps = psp.tile([128, 512], F32, tag="big")
                        exps = bp.tile([128, 512], BF16_DT, tag="exps")
                        nc.tensor.matmul(
                            lg_ps[:],
                            decstat[:, half * 128:(half + 1) * 128],
                            m_hist[:, msl],
                            start=True, stop=True)
                        nc.scalar.activation(exps[:], lg_ps[:], AF.Exp)
                        nc.tensor.matmul(se_ps[:], ones128[:], exps[:],
                                         start=(half == 0), stop=(half == 1))
                    lse_t = bp.tile([1, 512], F32, tag="lse")
                    nc.scalar.activation(lse_t[:], se_ps[:], AF.Ln)
                    paug_t = bp.tile([65, 512], BF16_DT, tag="paug")
                    nc.vector.tensor_tensor(paug_t[:], gaug[:, sl],
                                            m_hist[:, msl],
                                            mybir.AluOpType.mult)
                    pk_ps = psp.tile([1, 512], F32, tag="T_psA", bufs=1)
                    nc.tensor.matmul(pk_ps[:], ones65[:], paug_t[:],
                                     start=True, stop=True)
                    # loss = lse*inv_ln2 - pick (pick already scaled on host)
                    nc.vector.scalar_tensor_tensor(
                        loss[:, sl], lse_t[:], INV_LN2, pk_ps[:],
                        mybir.AluOpType.mult, mybir.AluOpType.subtract)
            nc.default_dma_engine.dma_start(out_d[:], loss[:])

    nc.compile()
    return nc


def _prep_core_inputs(x0, emb, W_enc_w, W_enc_b, W_dec_w, W_dec_b,
                      b_enc_w, b_enc_b, b_dec_w, b_dec_b, dec_w, dec_b):
    """Host-side gathers/packing -> list of per-core input dicts."""
    f32 = np.float32
    x0 = np.asarray(x0)
    xp = np.concatenate([np.zeros((B, L), x0.dtype), x0], axis=1)  # [B, N+L]
    e = np.asarray(emb, f32)[xp]  # [B, N+L, E]

    # shared weight packs
    Wcat = np.concatenate([np.asarray(W_enc_w, f32), np.asarray(b_enc_w, f32)],
                          axis=1)  # [1088, 128]
    wpre1 = np.zeros((128, 8 * 128), f32)
    wpre2 = np.zeros((128, 8 * 64), f32)
    for c in range(8):
        blk = Wcat[64 + 128 * c: 64 + 128 * (c + 1)]  # [128, 128]
        wpre1[:, c * 128: c * 128 + 64] = blk[:, :64]
        wpre1[:, c * 128 + 64: c * 128 + 128] = blk[:, :64]
        wpre2[:, c * 64:(c + 1) * 64] = blk[:, 64:]
    bias1 = np.concatenate([np.asarray(W_enc_b, f32)] * 2).reshape(1, 128)
    bias2 = np.asarray(b_enc_b, f32).reshape(1, 64)
    gse = np.concatenate([Wcat[:64, :64]] * 2, axis=1)    # [64, 128]
    gso = np.concatenate([Wcat[:64, 64:128]] * 2, axis=1)  # [64, 128]

    W2r = np.asarray(W_dec_w, f32).reshape(H, M, M)  # [h, i, j]
    wstatT = np.zeros((64, 32 * 128), f32)
    for c in range(32):
        wstatT[:, c * 128: c * 128 + 64] = W2r[:, c, :].T
        wstatT[:, c * 128 + 64: c * 128 + 128] = W2r[:, c + 32, :].T
    wbT = np.asarray(W_dec_b, f32).reshape(M, M).T.copy()  # [j, i]
    bw65 = np.concatenate([np.asarray(b_dec_w, f32),
                           np.asarray(b_dec_b, f32).reshape(1, 64)], axis=0)
    decstat = np.concatenate([np.asarray(dec_w, f32),
                              np.asarray(dec_b, f32).reshape(1, 256)], axis=0)
    ones65 = np.ones((65, 1), f32)
    ones128 = np.ones((128, 1), f32)

    shared = dict(
        wpre1=wpre1.astype(BF16), wpre2=wpre2.astype(BF16),
        bias1=bias1.astype(BF16), bias2=bias2.astype(BF16),
        gse=gse.astype(BF16), gso=gso.astype(BF16),
        wstatT=wstatT.astype(BF16), wbT=wbT.astype(BF16),
        bw65=bw65.astype(BF16),
        decstat=decstat.astype(BF16),
        ones65=ones65.astype(BF16), ones128=ones128.astype(BF16),
    )

    in_maps = []
    inv_ln2 = np.float32(1.0 / np.log(2.0))
    dec_wT = np.asarray(dec_w, f32).T.copy()  # [256, 64]
    dec_bv = np.asarray(dec_b, f32)
    for k in range(NCORES):
        rows = slice(BL * k, BL * (k + 1))
        ek = e[rows]  # [BL, N+L, E]
        # e8[l_sub*16+eps, tau*BL+b] = ek[b, tau+l_sub, eps]
        e8 = np.zeros((128, E8COLS), f32)
        for ls in range(8):
            blk = ek[:, ls: ls + TAU, :].transpose(2, 1, 0)  # [E, TAU, BL]
            e8[ls * 16:(ls + 1) * 16] = blk.reshape(E, E8COLS)
        y = np.asarray(x0[rows])  # [BL, N]
        g = dec_wT[y]  # [BL, N, 64]
        gaug = np.zeros((65, NB), f32)
        gaug[:64] = g.transpose(2, 1, 0).reshape(64, NB)
        gaug[64] = dec_bv[y].T.reshape(NB)
        gaug *= inv_ln2
        d = dict(shared)
        d["e8"] = e8.astype(BF16)
        d["gaug"] = gaug.astype(BF16)
        in_maps.append(d)
    return in_maps


def kernel(**inputs):
    key = "nc"
    if key not in _cache:
        _cache[key] = _build_nc()
    nc = _cache[key]
    in_maps = _prep_core_inputs(**inputs)
    res = run_bass_kernel_spmd(nc, in_maps, list(range(NCORES)),
                               trace=bool(os.environ.get("KERNEL_TRACE")))
    _cache["last_result"] = res
    out = np.zeros((N, B), np.float32)
    for k in range(NCORES):
        out[:, BL * k: BL * (k + 1)] = res.results[k]["out"].reshape(N, BL)
    return out.reshape(-1)


# revision 10
# speedup vs baseline: 1.0281x; 1.0281x over previous
"""Trainium2 Bass kernel for nn_Net4 (hypernetwork RNN scan), v2.

Model (per step t, per batch row b):
  h1 = sigmoid(m @ A1 + pre1[t])          A1 = W_enc_w[:64]
  h2 = sigmoid(m @ B1 + pre2[t])          B1 = b_enc_w[:64]
  Wm = (h1 @ W_dec_w + W_dec_b).reshape(64,64)
  bm = h2 @ b_dec_w + b_dec_b
  m' = sigmoid(Wm @ m + bm)
  loss[t] = (logsumexp(m'@dec_w+dec_b) - (m'@dec_w+dec_b)[y]) / ln2

v2 changes vs v1:
  - everything in the scan is bf16 (v1 had fp32 wbT/bw65/tsb matmuls that
    cost 2x LDWEIGHTS+MATMUL at ~200ns each)
  - pre1/pre2 are pre-written into the g PSUM bank by the vector engine
    (interleaved preg layout), and the g matmuls accumulate on top with
    start=False; this removes the vector add from the critical path
  - single m_bf [64,2]: all T-chunk stationaries live on partitions 0-63
    so no duplicated m halves and only one sigmoid per step boundary
  - m_hist is a bf16 copy of m_bf done by the vector engine (off the
    critical path), feeding a bf16 bulk loss phase (v1 bulk was fp32 and
    took ~106us; bf16 takes ~15us)

Sharding: batch rows 2k,2k+1 -> core k; zero cross-core communication.
"""

import os
import sys
import numpy as np

sys.path.insert(0, "/opt/trn_rl_repo")

import concourse.bass as bass
import concourse.bacc as bacc
import concourse.mybir as mybir
import concourse.tile as tile
from concourse.bass_utils import run_bass_kernel_spmd

import ml_dtypes

BF16 = ml_dtypes.bfloat16

Cin, E, L, M, H, Cout = 256, 16, 64, 64, 64, 256
B, N = 16, 2048
D = M + L * E  # 1088
NCORES = 8
BL = B // NCORES  # 2 batch rows per core
NB = N * BL       # 4096 (t,b) pairs per core
TAU = N + L - 8   # e8 time length: tau in [0, 2104)
E8COLS = TAU * BL  # 4208

F32 = mybir.dt.float32
BF16_DT = mybir.dt.bfloat16
AF = mybir.ActivationFunctionType
FP8_DT = mybir.dt.float8e4
FP8 = ml_dtypes.float8_e4m3fn
INV_LN2 = float(1.0 / np.log(2.0))

_cache = {}


def _build_nc(unroll=32):
    nc = bacc.Bacc("TRN2", target_bir_lowering=False, debug=True)

    # ---- DRAM parameters (per-core inputs) ----
    def P(name, shape, dt):
        return nc.declare_dram_parameter(name, list(shape), dt, isOutput=False)

    e8_d = P("e8", (128, E8COLS), BF16_DT)
    wpre1_d = P("wpre1", (128, 8 * 128), BF16_DT)
    wpre2_d = P("wpre2", (128, 8 * 64), BF16_DT)
    bias1_d = P("bias1", (1, 128), BF16_DT)   # [W_enc_b | W_enc_b]
    bias2_d = P("bias2", (1, 64), BF16_DT)    # b_enc_b
    gse_d = P("gse", (64, 128), BF16_DT)      # [A1 | A1]
    gso_d = P("gso", (64, 128), BF16_DT)      # [B1 | B1]
    wstatT_d = P("wstatT", (64, 32 * 128), BF16_DT)
    wbT_d = P("wbT", (64, 64), BF16_DT)       # W_dec_b reshaped [j,i]
    bw65_d = P("bw65", (65, 64), BF16_DT)     # [b_dec_w ; b_dec_b]
    decstat_d = P("decstat", (65, 256), BF16_DT)  # [dec_w ; dec_b]
    gaug_d = P("gaug", (65, NB), BF16_DT)     # picked dec cols * inv_ln2
    ones65_d = P("ones65", (65, 1), BF16_DT)
    ones128_d = P("ones128", (128, 1), BF16_DT)
    out_d = nc.declare_dram_parameter("out", [1, NB], F32, isOutput=True)

    with tile.TileContext(nc) as tc:
        with (
            tc.tile_pool(name="persist", bufs=1) as pp,
            tc.tile_pool(name="psum", bufs=2, space="PSUM") as psp,
        ):
            e8 = pp.tile([128, E8COLS], BF16_DT)
            wpre1 = pp.tile([128, 8 * 128], BF16_DT)
            wpre2 = pp.tile([128, 8 * 64], BF16_DT)
            bias1 = pp.tile([1, 128], BF16_DT)
            bias2 = pp.tile([1, 64], BF16_DT)
            gse = pp.tile([64, 128], BF16_DT)
            gso = pp.tile([64, 128], BF16_DT)
            wstatT = pp.tile([64, 32 * 128], BF16_DT)
            wbT = pp.tile([64, 64], BF16_DT)
            bw65 = pp.tile([65, 64], BF16_DT)
            decstat = pp.tile([65, 256], BF16_DT)
            gaug = pp.tile([65, NB], BF16_DT)
            ones65 = pp.tile([65, 1], BF16_DT)
            ones128 = pp.tile([128, 1], BF16_DT)

            for sb, dr in [
                (e8, e8_d), (wpre1, wpre1_d), (wpre2, wpre2_d),
                (bias1, bias1_d), (bias2, bias2_d),
                (gse, gse_d), (gso, gso_d),
                (wstatT, wstatT_d), (wbT, wbT_d),
                (bw65, bw65_d), (decstat, decstat_d),
                (gaug, gaug_d), (ones65, ones65_d), (ones128, ones128_d),
            ]:
                nc.default_dma_engine.dma_start(sb[:], dr[:])

            preg = pp.tile([128, N, 4], F32)      # pre1 (dup) | pre2 per t
            m_hist = pp.tile([65, NB + 2 * BL], BF16_DT)  # row 64 == 1.0
            m_bf = pp.tile([64, BL], BF16_DT)
            h1d = pp.tile([128, BL], BF16_DT)
            h2t = pp.tile([65, BL], BF16_DT)      # row 64 == 1.0
            tsb = pp.tile([128, 32, BL], BF16_DT)
            onerow = pp.tile([1, 512], BF16_DT)
            zeros4 = pp.tile([64, 4], BF16_DT)

            nc.vector.memset(m_hist[0:64, 0:BL], 0.0)
            nc.vector.memset(m_hist[64:65, :], 1.0)
            nc.vector.memset(m_bf[:], 0.0)
            nc.vector.memset(h2t[64:65, :], 1.0)
            nc.vector.memset(onerow[:], 1.0)
            nc.vector.memset(zeros4[:], 0.0)

            # ---- precompute preg = [pre1(b0),pre1(b1),pre2(b0),pre2(b1)] ----
            for n in range(8):
                ps1 = psp.tile([128, 256, BL], F32, tag="big")
                for c in range(8):
                    nc.tensor.matmul(
                        ps1[:],
                        wpre1[:, c * 128:(c + 1) * 128],
                        e8[:, 16 * c + 512 * n: 16 * c + 512 * n + 512],
                        start=(c == 0), stop=False,
                    )
                nc.tensor.matmul(ps1[:], bias1[:], onerow[:],
                                 start=False, stop=True, skip_group_check=True)
                nc.vector.tensor_copy(preg[:, 256 * n:256 * (n + 1), 0:2],
                                      ps1[:])
                ps2 = psp.tile([64, 256, BL], F32, tag="big")
                for c in range(8):
                    nc.tensor.matmul(
                        ps2[:],
                        wpre2[:, c * 64:(c + 1) * 64],
                        e8[:, 16 * c + 512 * n: 16 * c + 512 * n + 512],
                        start=(c == 0), stop=False,
                    )
                nc.tensor.matmul(ps2[:], bias2[:], onerow[:],
                                 start=False, stop=True, skip_group_check=True)
                nc.vector.tensor_copy(preg[0:64, 256 * n:256 * (n + 1), 2:4],
                                      ps2[:])

            # warm up the two g PSUM banks so their has_written bits are set
            # (the in-loop g matmuls use start=False over a DVE pre-write)
            for _ in range(2):
                gw = psp.tile([128, 1, 4], F32, tag="g_ps")
                nc.tensor.matmul(gw[:, 0, :], gse[:], zeros4[:],
                                 start=True, stop=True)
            # dummy sigmoid: loads the act table on the preamble path so the
            # fixpoint pass hoists the per-iteration ACT_TABLE_LOAD (1283ns)
            # out of the scan loop
            sigscratch = pp.tile([1, 1], F32)
            nc.scalar.activation(sigscratch[:], zeros4[0:1, 0:1], AF.Sigmoid)

            # ---- the scan ----
            with tc.For_i(0, N, unroll, staggered_reset=True,
                          hint_engines=(mybir.EngineType.PE,)) as iv:
                for k in range(unroll):
                    t = iv + k
                    tcol = t * BL
                    g_ps = psp.tile([128, 1, 4], F32, tag="g_ps")
                    a_ps = psp.tile([64, BL], F32, tag="a_ps", bufs=1)
                    T_psA = psp.tile([128, 16, BL], F32, tag="T_psA", bufs=1)
                    T_psB = psp.tile([128, 16, BL], F32, tag="T_psB", bufs=1)

                    # pre1/pre2 pre-written into the g bank (vector, early)
                    nc.vector.tensor_copy(g_ps[:], preg[:, bass.ds(t, 1), :])

                    # g = [A1|A1]^T m (+pre1), [B1|B1]^T m (+pre2)
                    nc.tensor.matmul(g_ps[:, 0, 0:2], gse[:], m_bf[:],
                                     start=False, stop=True,
                                     skip_group_check=True)
                    nc.tensor.matmul(g_ps[:, 0, 2:4], gso[:], m_bf[:],
                                     start=False, stop=True,
                                     skip_group_check=True)
                    # T chunks: T[p, c, b]: p<64 -> T[h=p, i=c],
                    #                       p>=64 -> T[h=p-64, i=c+32]
                    for c in range(16):
                        nc.tensor.matmul(
                            T_psA[:, c, :],
                            wstatT[:, c * 128:(c + 1) * 128],
                            m_bf[:], start=True, stop=True)
                    for c in range(16, 32):
                        nc.tensor.matmul(
                            T_psB[:, c - 16, :],
                            wstatT[:, c * 128:(c + 1) * 128],
                            m_bf[:], start=True, stop=True)

                    # h = sigmoid(g) (g already includes pre via pre-write)
                    nc.scalar.activation(h1d[:], g_ps[:, 0, 0:2], AF.Sigmoid)
                    nc.scalar.activation(h2t[0:64, :], g_ps[0:64, 0, 2:4],
                                         AF.Sigmoid)

                    # T -> SBUF (bf16); A copy overlaps the B-chunk matmuls
                    nc.vector.tensor_copy(tsb[:, 0:16, :], T_psA[:])
                    nc.vector.tensor_copy(tsb[:, 16:32, :], T_psB[:])

                    # a = WbT@m + bw65@[h2;1] + sum_h h1*T
                    nc.tensor.matmul(a_ps[:], wbT[:], m_bf[:],
                                     start=True, stop=False)
                    nc.tensor.matmul(a_ps[:], bw65[:], h2t[:],
                                     start=False, stop=False,
                                     skip_group_check=True)
                    for b in range(BL):
                        nc.tensor.matmul(a_ps[0:32, b: b + 1],
                                         tsb[0:64, :, b], h1d[0:64, b: b + 1],
                                         start=False, stop=False,
                                         skip_group_check=True,
                                         tile_position=(0, 0))
                        last = b == BL - 1
                        nc.tensor.matmul(a_ps[32:64, b: b + 1],
                                         tsb[64:128, :, b],
                                         h1d[64:128, b: b + 1],
                                         start=False, stop=last,
                                         skip_group_check=True,
                                         tile_position=(64, 32))

                    # m' = sigmoid(a)
                    nc.scalar.activation(m_bf[:], a_ps[:], AF.Sigmoid)
                    nc.vector.tensor_copy(
                        m_hist[0:64, bass.ds(tcol + BL, BL)], m_bf[:])

            # ---- bulk loss ----
            loss = pp.tile([1, NB], F32)
            with tc.tile_pool(name="bulk", bufs=2) as bp:
                for tcn in range(8):
                    sl = slice(512 * tcn, 512 * (tcn + 1))
                    msl = slice(BL + 512 * tcn, BL + 512 * (tcn + 1))
                    se_ps = psp.tile([1, 512], F32, tag="seps", bufs=1)
                    for half in range(2):
                        lg_ps = psp.tile([128, 512], F32, tag="big")
                        exps = bp.tile([128, 512], BF16_DT, tag="exps")
                        nc.tensor.matmul(
                            lg_ps[:],
                            decstat[:, half * 128:(half + 1) * 128],
                            m_hist[:, msl],
                            start=True, stop=True)
                        nc.scalar.activation(exps[:], lg_ps[:], AF.Exp)
                        nc.tensor.matmul(se_ps[:], ones128[:], exps[:],
                                         start=(half == 0), stop=(half == 1))
                    lse_t = bp.tile([1, 512], F32, tag="lse")
                    nc.scalar.activation(lse_t[:], se_ps[:], AF.Ln)
                    paug_t = bp.tile([65, 512], BF16_DT, tag="paug")
                    nc.vector.tensor_tensor(paug_t[:], gaug[:, sl],
                                            m_hist[:, msl],
                                            mybir.AluOpType.mult)
                    pk_ps = psp.tile([1, 512], F32, tag="T_psA", bufs=1)
                    nc.tensor.matmul(pk_ps[:], ones65[:], paug_t[:],
                                     start=True, stop=True)
                    # loss = lse*inv_ln2 - pick (pick already scaled on host)
                    nc.vector.scalar_tensor_tensor(
                        loss[:, sl], lse_t[:], INV_LN2, pk_ps[:],
                        mybir.AluOpType.mult, mybir.AluOpType.subtract)
            nc.default_dma_engine.dma_start(out_d[:], loss[:])

    nc.compile()
    return nc


def _prep_core_inputs(x0, emb, W_enc_w, W_enc_b, W_dec_w, W_dec_b,
                      b_enc_w, b_enc_b, b_dec_w, b_dec_b, dec_w, dec_b):
    """Host-side gathers/packing -> list of per-core input dicts."""
    f32 = np.float32
    x0 = np.asarray(x0)
    xp = np.concatenate([np.zeros((B, L), x0.dtype), x0], axis=1)  # [B, N+L]
    e = np.asarray(emb, f32)[xp]  # [B, N+L, E]

    # shared weight packs
    Wcat = np.concatenate([np.asarray(W_enc_w, f32), np.asarray(b_enc_w, f32)],
                          axis=1)  # [1088, 128]
    wpre1 = np.zeros((128, 8 * 128), f32)
    wpre2 = np.zeros((128, 8 * 64), f32)
    for c in range(8):
        blk = Wcat[64 + 128 * c: 64 + 128 * (c + 1)]  # [128, 128]
        wpre1[:, c * 128: c * 128 + 64] = blk[:, :64]
        wpre1[:, c * 128 + 64: c * 128 + 128] = blk[:, :64]
        wpre2[:, c * 64:(c + 1) * 64] = blk[:, 64:]
    bias1 = np.concatenate([np.asarray(W_enc_b, f32)] * 2).reshape(1, 128)
    bias2 = np.asarray(b_enc_b, f32).reshape(1, 64)
    gse = np.concatenate([Wcat[:64, :64]] * 2, axis=1)    # [64, 128]
    gso = np.concatenate([Wcat[:64, 64:128]] * 2, axis=1)  # [64, 128]

    W2r = np.asarray(W_dec_w, f32).reshape(H, M, M)  # [h, i, j]
    wstatT = np.zeros((64, 32 * 128), f32)
    for c in range(32):
        wstatT[:, c * 128: c * 128 + 64] = W2r[:, c, :].T
        wstatT[:, c * 128 + 64: c * 128 + 128] = W2r[:, c + 32, :].T
    wbT = np.asarray(W_dec_b, f32).reshape(M, M).T.copy()  # [j, i]
    bw65 = np.concatenate([np.asarray(b_dec_w, f32),
                           np.asarray(b_dec_b, f32).reshape(1, 64)], axis=0)
    decstat = np.concatenate([np.asarray(dec_w, f32),
                              np.asarray(dec_b, f32).reshape(1, 256)], axis=0)
    ones65 = np.ones((65, 1), f32)
    ones128 = np.ones((128, 1), f32)

    shared = dict(
        wpre1=wpre1.astype(BF16), wpre2=wpre2.astype(BF16),
        bias1=bias1.astype(BF16), bias2=bias2.astype(BF16),
        gse=gse.astype(BF16), gso=gso.astype(BF16),
        wstatT=wstatT.astype(BF16), wbT=wbT.astype(BF16),
        bw65=bw65.astype(BF16),
        decstat=decstat.astype(BF16),
        ones65=ones65.astype(BF16), ones128=ones128.astype(BF16),
    )

    in_maps = []
    inv_ln2 = np.float32(1.0 / np.log(2.0))
    dec_wT = np.asarray(dec_w, f32).T.copy()  # [256, 64]
    dec_bv = np.asarray(dec_b, f32)
    for k in range(NCORES):
        rows = slice(BL * k, BL * (k + 1))
        ek = e[rows]  # [BL, N+L, E]
        # e8[l_sub*16+eps, tau*BL+b] = ek[b, tau+l_sub, eps]
        e8 = np.zeros((128, E8COLS), f32)
        for ls in range(8):
            blk = ek[:, ls: ls + TAU, :].transpose(2, 1, 0)  # [E, TAU, BL]
            e8[ls * 16:(ls + 1) * 16] = blk.reshape(E, E8COLS)
        y = np.asarray(x0[rows])  # [BL, N]
        g = dec_wT[y]  # [BL, N, 64]
        gaug = np.zeros((65, NB), f32)
        gaug[:64] = g.transpose(2, 1, 0).reshape(64, NB)
        gaug[64] = dec_bv[y].T.reshape(NB)
        gaug *= inv_ln2
        d = dict(shared)
        d["e8"] = e8.astype(BF16)
        d["gaug"] = gaug.astype(BF16)
        in_maps.append(d)
    return in_maps


def kernel(**inputs):
    key = "nc"
    if key not in _cache:
        _cache[key] = _build_nc()
    nc = _cache[key]
    in_maps = _prep_core_inputs(**inputs)
    res = run_bass_kernel_spmd(nc, in_maps, list(range(NCORES)),
                               trace=bool(os.environ.get("KERNEL_TRACE")))
    _cache["last_result"] = res
    out = np.zeros((N, B), np.float32)
    for k in range(NCORES):
        out[:, BL * k: BL * (k + 1)] = res.results[k]["out"].reshape(N, BL)
    return out.reshape(-1)


# revision 11
# speedup vs baseline: 1.0296x; 1.0014x over previous
"""Trainium2 Bass kernel for nn_Net4 (hypernetwork RNN scan), v2.

Model (per step t, per batch row b):
  h1 = sigmoid(m @ A1 + pre1[t])          A1 = W_enc_w[:64]
  h2 = sigmoid(m @ B1 + pre2[t])          B1 = b_enc_w[:64]
  Wm = (h1 @ W_dec_w + W_dec_b).reshape(64,64)
  bm = h2 @ b_dec_w + b_dec_b
  m' = sigmoid(Wm @ m + bm)
  loss[t] = (logsumexp(m'@dec_w+dec_b) - (m'@dec_w+dec_b)[y]) / ln2

v2 changes vs v1:
  - everything in the scan is bf16 (v1 had fp32 wbT/bw65/tsb matmuls that
    cost 2x LDWEIGHTS+MATMUL at ~200ns each)
  - pre1/pre2 are pre-written into the g PSUM bank by the vector engine
    (interleaved preg layout), and the g matmuls accumulate on top with
    start=False; this removes the vector add from the critical path
  - single m_bf [64,2]: all T-chunk stationaries live on partitions 0-63
    so no duplicated m halves and only one sigmoid per step boundary
  - m_hist is a bf16 copy of m_bf done by the vector engine (off the
    critical path), feeding a bf16 bulk loss phase (v1 bulk was fp32 and
    took ~106us; bf16 takes ~15us)

Sharding: batch rows 2k,2k+1 -> core k; zero cross-core communication.
"""

import os
import sys
import numpy as np

sys.path.insert(0, "/opt/trn_rl_repo")

import concourse.bass as bass
import concourse.bacc as bacc
import concourse.mybir as mybir
import concourse.tile as tile
from concourse.bass_utils import run_bass_kernel_spmd

import ml_dtypes

BF16 = ml_dtypes.bfloat16

Cin, E, L, M, H, Cout = 256, 16, 64, 64, 64, 256
B, N = 16, 2048
D = M + L * E  # 1088
NCORES = 8
BL = B // NCORES  # 2 batch rows per core
NB = N * BL       # 4096 (t,b) pairs per core
TAU = N + L - 8   # e8 time length: tau in [0, 2104)
E8COLS = TAU * BL  # 4208

F32 = mybir.dt.float32
BF16_DT = mybir.dt.bfloat16
AF = mybir.ActivationFunctionType
FP8_DT = mybir.dt.float8e4
FP8 = ml_dtypes.float8_e4m3fn
INV_LN2 = float(1.0 / np.log(2.0))

_cache = {}


def _build_nc(unroll=32):
    nc = bacc.Bacc("TRN2", target_bir_lowering=False, debug=True)

    # ---- DRAM parameters (per-core inputs) ----
    def P(name, shape, dt):
        return nc.declare_dram_parameter(name, list(shape), dt, isOutput=False)

    e8_d = P("e8", (128, E8COLS), BF16_DT)
    wpre1_d = P("wpre1", (128, 8 * 128), BF16_DT)
    wpre2_d = P("wpre2", (128, 8 * 64), BF16_DT)
    bias1_d = P("bias1", (1, 128), BF16_DT)   # [W_enc_b | W_enc_b]
    bias2_d = P("bias2", (1, 64), BF16_DT)    # b_enc_b
    gse_d = P("gse", (64, 128), BF16_DT)      # [A1 | A1]
    gso_d = P("gso", (64, 128), BF16_DT)      # [B1 | B1]
    wsing_d = P("wsing", (64, 8 * 128), BF16_DT)
    wpair_d = P("wpair", (128, 12 * 128), BF16_DT)
    wbT_d = P("wbT", (64, 64), BF16_DT)       # W_dec_b reshaped [j,i]
    bw65_d = P("bw65", (65, 64), BF16_DT)     # [b_dec_w ; b_dec_b]
    decstat_d = P("decstat", (65, 256), BF16_DT)  # [dec_w ; dec_b]
    gaug_d = P("gaug", (65, NB), BF16_DT)     # picked dec cols * inv_ln2
    ones65_d = P("ones65", (65, 1), BF16_DT)
    ones128_d = P("ones128", (128, 1), BF16_DT)
    out_d = nc.declare_dram_parameter("out", [1, NB], F32, isOutput=True)

    with tile.TileContext(nc) as tc:
        with (
            tc.tile_pool(name="persist", bufs=1) as pp,
            tc.tile_pool(name="psum", bufs=2, space="PSUM") as psp,
        ):
            e8 = pp.tile([128, E8COLS], BF16_DT)
            wpre1 = pp.tile([128, 8 * 128], BF16_DT)
            wpre2 = pp.tile([128, 8 * 64], BF16_DT)
            bias1 = pp.tile([1, 128], BF16_DT)
            bias2 = pp.tile([1, 64], BF16_DT)
            gse = pp.tile([64, 128], BF16_DT)
            gso = pp.tile([64, 128], BF16_DT)
            wsing = pp.tile([64, 8 * 128], BF16_DT)
            wpair = pp.tile([128, 12 * 128], BF16_DT)
            wbT = pp.tile([64, 64], BF16_DT)
            bw65 = pp.tile([65, 64], BF16_DT)
            decstat = pp.tile([65, 256], BF16_DT)
            gaug = pp.tile([65, NB], BF16_DT)
            ones65 = pp.tile([65, 1], BF16_DT)
            ones128 = pp.tile([128, 1], BF16_DT)

            for sb, dr in [
                (e8, e8_d), (wpre1, wpre1_d), (wpre2, wpre2_d),
                (bias1, bias1_d), (bias2, bias2_d),
                (gse, gse_d), (gso, gso_d),
                (wsing, wsing_d), (wpair, wpair_d), (wbT, wbT_d),
                (bw65, bw65_d), (decstat, decstat_d),
                (gaug, gaug_d), (ones65, ones65_d), (ones128, ones128_d),
            ]:
                nc.default_dma_engine.dma_start(sb[:], dr[:])

            preg = pp.tile([128, N, 4], F32)      # pre1 (dup) | pre2 per t
            m_hist = pp.tile([65, NB + 2 * BL], BF16_DT)  # row 64 == 1.0
            m4 = pp.tile([128, 4], BF16_DT)
            h1d = pp.tile([128, BL], BF16_DT)
            h2t = pp.tile([65, BL], BF16_DT)      # row 64 == 1.0
            tsb = pp.tile([128, 32, BL], BF16_DT)
            onerow = pp.tile([1, 512], BF16_DT)
            zeros4 = pp.tile([64, 4], BF16_DT)

            nc.vector.memset(m_hist[0:64, 0:BL], 0.0)
            nc.vector.memset(m_hist[64:65, :], 1.0)
            nc.vector.memset(m4[:], 0.0)
            nc.vector.memset(h2t[64:65, :], 1.0)
            nc.vector.memset(onerow[:], 1.0)
            nc.vector.memset(zeros4[:], 0.0)

            # ---- precompute preg = [pre1(b0),pre1(b1),pre2(b0),pre2(b1)] ----
            for n in range(8):
                ps1 = psp.tile([128, 256, BL], F32, tag="big")
                for c in range(8):
                    nc.tensor.matmul(
                        ps1[:],
                        wpre1[:, c * 128:(c + 1) * 128],
                        e8[:, 16 * c + 512 * n: 16 * c + 512 * n + 512],
                        start=(c == 0), stop=False,
                    )
                nc.tensor.matmul(ps1[:], bias1[:], onerow[:],
                                 start=False, stop=True, skip_group_check=True)
                nc.vector.tensor_copy(preg[:, 256 * n:256 * (n + 1), 0:2],
                                      ps1[:])
                ps2 = psp.tile([64, 256, BL], F32, tag="big")
                for c in range(8):
                    nc.tensor.matmul(
                        ps2[:],
                        wpre2[:, c * 64:(c + 1) * 64],
                        e8[:, 16 * c + 512 * n: 16 * c + 512 * n + 512],
                        start=(c == 0), stop=False,
                    )
                nc.tensor.matmul(ps2[:], bias2[:], onerow[:],
                                 start=False, stop=True, skip_group_check=True)
                nc.vector.tensor_copy(preg[0:64, 256 * n:256 * (n + 1), 2:4],
                                      ps2[:])

            # warm up the two g PSUM banks so their has_written bits are set
            # (the in-loop g matmuls use start=False over a DVE pre-write)
            for _ in range(2):
                gw = psp.tile([128, 1, 4], F32, tag="g_ps")
                nc.tensor.matmul(gw[:, 0, :], gse[:], zeros4[:],
                                 start=True, stop=True)
            # dummy sigmoid: loads the act table on the preamble path so the
            # fixpoint pass hoists the per-iteration ACT_TABLE_LOAD (1283ns)
            # out of the scan loop
            sigscratch = pp.tile([1, 1], F32)
            nc.scalar.activation(sigscratch[:], zeros4[0:1, 0:1], AF.Sigmoid)

            # ---- the scan ----
            with tc.For_i(0, N, unroll,
                          hint_engines=(mybir.EngineType.PE,)) as iv:
                for k in range(unroll):
                    t = iv + k
                    tcol = t * BL
                    g_ps = psp.tile([128, 1, 4], F32, tag="g_ps")
                    a_ps = psp.tile([64, BL], F32, tag="a_ps", bufs=1)
                    T_psA = psp.tile([128, 16, BL], F32, tag="T_psA", bufs=1)
                    T_psB = psp.tile([128, 16, BL], F32, tag="T_psB", bufs=1)

                    # pre1/pre2 pre-written into the g bank (vector, early)
                    nc.vector.tensor_copy(g_ps[:], preg[:, bass.ds(t, 1), :])

                    # g = [A1|A1]^T m (+pre1), [B1|B1]^T m (+pre2)
                    nc.tensor.matmul(g_ps[:, 0, 0:2], gse[:], m4[0:64, 0:2],
                                     start=False, stop=True,
                                     skip_group_check=True)
                    nc.tensor.matmul(g_ps[:, 0, 2:4], gso[:], m4[0:64, 0:2],
                                     start=False, stop=True,
                                     skip_group_check=True)
                    # T chunks: T[p, c, b]: p<64 -> T[h=p, i=c],
                    #                       p>=64 -> T[h=p-64, i=c+32]
                    # chunks 0-7: single [64,128] stationaries, need only the
                    # lower half of m4 (run while sig#2 fills the upper half)
                    for c in range(8):
                        nc.tensor.matmul(
                            T_psA[:, c, :],
                            wsing[:, c * 128:(c + 1) * 128],
                            m4[0:64, 0:2], start=True, stop=True)
                    # chunks 8-31: [128,128] pair stationaries, 4-col moving
                    # [m;0 | 0;m] computes two chunks per LDWEIGHTS
                    for p in range(4):
                        nc.tensor.matmul(
                            T_psA[:, 8 + 2 * p: 10 + 2 * p, :],
                            wpair[:, p * 128:(p + 1) * 128],
                            m4[:], start=True, stop=True)
                    for p in range(4, 12):
                        nc.tensor.matmul(
                            T_psB[:, 2 * p - 8: 2 * p - 6, :],
                            wpair[:, p * 128:(p + 1) * 128],
                            m4[:], start=True, stop=True)

                    # h = sigmoid(g) (g already includes pre via pre-write)
                    nc.scalar.activation(h1d[:], g_ps[:, 0, 0:2], AF.Sigmoid)
                    nc.scalar.activation(h2t[0:64, :], g_ps[0:64, 0, 2:4],
                                         AF.Sigmoid)

                    # T -> SBUF (bf16); A copy overlaps the B-chunk matmuls
                    nc.vector.tensor_copy(tsb[:, 0:16, :], T_psA[:])
                    nc.vector.tensor_copy(tsb[:, 16:32, :], T_psB[:])

                    # a = WbT@m + bw65@[h2;1] + sum_h h1*T
                    nc.tensor.matmul(a_ps[:], wbT[:], m4[0:64, 0:2],
                                     start=True, stop=False)
                    nc.tensor.matmul(a_ps[:], bw65[:], h2t[:],
                                     start=False, stop=False,
                                     skip_group_check=True)
                    for b in range(BL):
                        nc.tensor.matmul(a_ps[0:32, b: b + 1],
                                         tsb[0:64, :, b], h1d[0:64, b: b + 1],
                                         start=False, stop=False,
                                         skip_group_check=True,
                                         tile_position=(0, 0))
                        last = b == BL - 1
                        nc.tensor.matmul(a_ps[32:64, b: b + 1],
                                         tsb[64:128, :, b],
                                         h1d[64:128, b: b + 1],
                                         start=False, stop=last,
                                         skip_group_check=True,
                                         tile_position=(64, 32))

                    # m' = sigmoid(a): lower half first (gates g + single
                    # chunks), upper copy of it second (gates pair chunks)
                    nc.scalar.activation(m4[0:64, 0:2], a_ps[:], AF.Sigmoid)
                    nc.scalar.activation(m4[64:128, 2:4], a_ps[:], AF.Sigmoid)
                    nc.vector.tensor_copy(
                        m_hist[0:64, bass.ds(tcol + BL, BL)], m4[0:64, 0:2])

            # ---- bulk loss ----
            loss = pp.tile([1, NB], F32)
            with tc.tile_pool(name="bulk", bufs=2) as bp:
                for tcn in range(8):
                    sl = slice(512 * tcn, 512 * (tcn + 1))
                    msl = slice(BL + 512 * tcn, BL + 512 * (tcn + 1))
                    se_ps = psp.tile([1, 512], F32, tag="seps", bufs=1)
                    for half in range(2):
                        lg_ps = psp.tile([128, 512], F32, tag="big")
                        exps = bp.tile([128, 512], BF16_DT, tag="exps")
                        nc.tensor.matmul(
                            lg_ps[:],
                            decstat[:, half * 128:(half + 1) * 128],
                            m_hist[:, msl],
                            start=True, stop=True)
                        nc.scalar.activation(exps[:], lg_ps[:], AF.Exp)
                        nc.tensor.matmul(se_ps[:], ones128[:], exps[:],
                                         start=(half == 0), stop=(half == 1))
                    lse_t = bp.tile([1, 512], F32, tag="lse")
                    nc.scalar.activation(lse_t[:], se_ps[:], AF.Ln)
                    paug_t = bp.tile([65, 512], BF16_DT, tag="paug")
                    nc.vector.tensor_tensor(paug_t[:], gaug[:, sl],
                                            m_hist[:, msl],
                                            mybir.AluOpType.mult)
                    pk_ps = psp.tile([1, 512], F32, tag="T_psA", bufs=1)
                    nc.tensor.matmul(pk_ps[:], ones65[:], paug_t[:],
                                     start=True, stop=True)
                    # loss = lse*inv_ln2 - pick (pick already scaled on host)
                    nc.vector.scalar_tensor_tensor(
                        loss[:, sl], lse_t[:], INV_LN2, pk_ps[:],
                        mybir.AluOpType.mult, mybir.AluOpType.subtract)
            nc.default_dma_engine.dma_start(out_d[:], loss[:])

    nc.compile()
    return nc


def _prep_core_inputs(x0, emb, W_enc_w, W_enc_b, W_dec_w, W_dec_b,
                      b_enc_w, b_enc_b, b_dec_w, b_dec_b, dec_w, dec_b):
    """Host-side gathers/packing -> list of per-core input dicts."""
    f32 = np.float32
    x0 = np.asarray(x0)
    xp = np.concatenate([np.zeros((B, L), x0.dtype), x0], axis=1)  # [B, N+L]
    e = np.asarray(emb, f32)[xp]  # [B, N+L, E]

    # shared weight packs
    Wcat = np.concatenate([np.asarray(W_enc_w, f32), np.asarray(b_enc_w, f32)],
                          axis=1)  # [1088, 128]
    wpre1 = np.zeros((128, 8 * 128), f32)
    wpre2 = np.zeros((128, 8 * 64), f32)
    for c in range(8):
        blk = Wcat[64 + 128 * c: 64 + 128 * (c + 1)]  # [128, 128]
        wpre1[:, c * 128: c * 128 + 64] = blk[:, :64]
        wpre1[:, c * 128 + 64: c * 128 + 128] = blk[:, :64]
        wpre2[:, c * 64:(c + 1) * 64] = blk[:, 64:]
    bias1 = np.concatenate([np.asarray(W_enc_b, f32)] * 2).reshape(1, 128)
    bias2 = np.asarray(b_enc_b, f32).reshape(1, 64)
    gse = np.concatenate([Wcat[:64, :64]] * 2, axis=1)    # [64, 128]
    gso = np.concatenate([Wcat[:64, 64:128]] * 2, axis=1)  # [64, 128]

    W2r = np.asarray(W_dec_w, f32).reshape(H, M, M)  # [h, i, j]
    wstatT = np.zeros((64, 32 * 128), f32)
    for c in range(32):
        wstatT[:, c * 128: c * 128 + 64] = W2r[:, c, :].T
        wstatT[:, c * 128 + 64: c * 128 + 128] = W2r[:, c + 32, :].T
    wbT = np.asarray(W_dec_b, f32).reshape(M, M).T.copy()  # [j, i]
    bw65 = np.concatenate([np.asarray(b_dec_w, f32),
                           np.asarray(b_dec_b, f32).reshape(1, 64)], axis=0)
    decstat = np.concatenate([np.asarray(dec_w, f32),
                              np.asarray(dec_b, f32).reshape(1, 256)], axis=0)
    ones65 = np.ones((65, 1), f32)
    ones128 = np.ones((128, 1), f32)

    wsing = wstatT[:, 0: 8 * 128]
    wpair = np.zeros((128, 12 * 128), f32)
    for p in range(12):
        wpair[0:64, p * 128:(p + 1) * 128] = \
            wstatT[:, (8 + 2 * p) * 128:(9 + 2 * p) * 128]
        wpair[64:128, p * 128:(p + 1) * 128] = \
            wstatT[:, (9 + 2 * p) * 128:(10 + 2 * p) * 128]
    shared = dict(
        wpre1=wpre1.astype(BF16), wpre2=wpre2.astype(BF16),
        bias1=bias1.astype(BF16), bias2=bias2.astype(BF16),
        gse=gse.astype(BF16), gso=gso.astype(BF16),
        wsing=wsing.astype(BF16), wpair=wpair.astype(BF16),
        wbT=wbT.astype(BF16),
        bw65=bw65.astype(BF16),
        decstat=decstat.astype(BF16),
        ones65=ones65.astype(BF16), ones128=ones128.astype(BF16),
    )

    in_maps = []
    inv_ln2 = np.float32(1.0 / np.log(2.0))
    dec_wT = np.asarray(dec_w, f32).T.copy()  # [256, 64]
    dec_bv = np.asarray(dec_b, f32)
    for k in range(NCORES):
        rows = slice(BL * k, BL * (k + 1))
        ek = e[rows]  # [BL, N+L, E]
        # e8[l_sub*16+eps, tau*BL+b] = ek[b, tau+l_sub, eps]
        e8 = np.zeros((128, E8COLS), f32)
        for ls in range(8):
            blk = ek[:, ls: ls + TAU, :].transpose(2, 1, 0)  # [E, TAU, BL]
            e8[ls * 16:(ls + 1) * 16] = blk.reshape(E, E8COLS)
        y = np.asarray(x0[rows])  # [BL, N]
        g = dec_wT[y]  # [BL, N, 64]
        gaug = np.zeros((65, NB), f32)
        gaug[:64] = g.transpose(2, 1, 0).reshape(64, NB)
        gaug[64] = dec_bv[y].T.reshape(NB)
        gaug *= inv_ln2
        d = dict(shared)
        d["e8"] = e8.astype(BF16)
        d["gaug"] = gaug.astype(BF16)
        in_maps.append(d)
    return in_maps


def kernel(**inputs):
    key = "nc"
    if key not in _cache:
        _cache[key] = _build_nc()
    nc = _cache[key]
    in_maps = _prep_core_inputs(**inputs)
    res = run_bass_kernel_spmd(nc, in_maps, list(range(NCORES)),
                               trace=bool(os.environ.get("KERNEL_TRACE")))
    _cache["last_result"] = res
    out = np.zeros((N, B), np.float32)
    for k in range(NCORES):
        out[:, BL * k: BL * (k + 1)] = res.results[k]["out"].reshape(N, BL)
    return out.reshape(-1)


# revision 24
# speedup vs baseline: 1.2247x; 1.1896x over previous
"""Trainium2 Bass kernel for nn_Net4 (hypernetwork RNN scan), v2.

Model (per step t, per batch row b):
  h1 = sigmoid(m @ A1 + pre1[t])          A1 = W_enc_w[:64]
  h2 = sigmoid(m @ B1 + pre2[t])          B1 = b_enc_w[:64]
  Wm = (h1 @ W_dec_w + W_dec_b).reshape(64,64)
  bm = h2 @ b_dec_w + b_dec_b
  m' = sigmoid(Wm @ m + bm)
  loss[t] = (logsumexp(m'@dec_w+dec_b) - (m'@dec_w+dec_b)[y]) / ln2

Final design (7.24ms baseline -> 4.42ms, rel err 4.0e-4):
  - everything in the scan is bf16 (v1 had fp32 wbT/bw65/tsb matmuls that
    lower to 2x LDWEIGHTS+MATMUL LOW/HIGH pairs at ~200ns each)
  - pre1/pre2 are pre-written into the g PSUM bank by the vector engine
    (interleaved preg layout), and the g matmuls accumulate on top with
    start=False (after a PE warmup that sets the has_written bits); this
    removes the vector add from the critical path
  - m4 [128,4] = [m;0 | 0;m]: only one sigmoid (lower half) on the step
    boundary; the upper half is a partition-shift Act copy emitted AFTER
    the single-chunk matmuls of the next step so they never wait on it
  - T block: 8 single [64,128] chunk stationaries (lower-half m only,
    run during the upper-half copy) + 12 paired [128,128] stationaries
    whose 4-col moving [m;0|0;m] yields two chunks per LDWEIGHTS
    (zero blocks kill the cross terms); more pairs is a net loss since
    a [128,128] LDWEIGHTS costs ~2x a [64,128] one
  - T PSUM split into two tiles so the first copy to SBUF overlaps the
    second half of the T matmuls (Tile dependency tracking is per-tile)
  - a dummy sigmoid before the loop keeps the act-table load out of the
    loop body (it cost 1283ns per For_i iteration otherwise)
  - m_hist is a bf16 copy of m done by the vector engine (off the
    critical path), feeding a bf16 bulk loss phase (v1 bulk was fp32 and
    took ~106us)
  - For_i unroll=32; each loop boundary still costs ~5us (all-engine
    barrier + semaphore resets)

Sharding: batch rows 2k,2k+1 -> core k; zero cross-core communication.
"""

import os
import sys
import numpy as np

sys.path.insert(0, "/opt/trn_rl_repo")

import concourse.bass as bass
import concourse.bacc as bacc
import concourse.mybir as mybir
import concourse.tile as tile
from concourse.bass_utils import run_bass_kernel_spmd

import ml_dtypes

BF16 = ml_dtypes.bfloat16

Cin, E, L, M, H, Cout = 256, 16, 64, 64, 64, 256
B, N = 16, 2048
D = M + L * E  # 1088
NCORES = 8
BL = B // NCORES  # 2 batch rows per core
NB = N * BL       # 4096 (t,b) pairs per core
TAU = N + L - 8   # e8 time length: tau in [0, 2104)
E8COLS = TAU * BL  # 4208

F32 = mybir.dt.float32
BF16_DT = mybir.dt.bfloat16
AF = mybir.ActivationFunctionType
FP8_DT = mybir.dt.float8e4
FP8 = ml_dtypes.float8_e4m3fn
INV_LN2 = float(1.0 / np.log(2.0))

_cache = {}


def _build_nc(unroll=128):
    nc = bacc.Bacc("TRN2", target_bir_lowering=False, debug=True)

    # ---- DRAM parameters (per-core inputs) ----
    def P(name, shape, dt):
        return nc.declare_dram_parameter(name, list(shape), dt, isOutput=False)

    e8_d = P("e8", (128, E8COLS), BF16_DT)
    wpre1_d = P("wpre1", (128, 8 * 128), BF16_DT)
    wpre2_d = P("wpre2", (128, 8 * 64), BF16_DT)
    bias1_d = P("bias1", (1, 128), BF16_DT)   # [W_enc_b | W_enc_b]
    bias2_d = P("bias2", (1, 64), BF16_DT)    # b_enc_b
    gse_d = P("gse", (64, 128), BF16_DT)      # [A1 | A1]
    gso_d = P("gso", (64, 128), BF16_DT)      # [B1 | B1]
    wsing_d = P("wsing", (64, 8 * 128), BF16_DT)
    wpair_d = P("wpair", (128, 12 * 128), BF16_DT)
    wbT_d = P("wbT", (64, 64), BF16_DT)       # W_dec_b reshaped [j,i]
    bw65_d = P("bw65", (65, 64), BF16_DT)     # [b_dec_w ; b_dec_b]
    decstat_d = P("decstat", (65, 256), BF16_DT)  # [dec_w ; dec_b]
    gaug_d = P("gaug", (65, NB), BF16_DT)     # picked dec cols * inv_ln2
    ones65_d = P("ones65", (65, 1), BF16_DT)
    ones128_d = P("ones128", (128, 1), BF16_DT)
    out_d = nc.declare_dram_parameter("out", [1, NB], F32, isOutput=True)

    with tile.TileContext(nc) as tc:
        with (
            tc.tile_pool(name="persist", bufs=1) as pp,
            tc.tile_pool(name="psum", bufs=2, space="PSUM") as psp,
        ):
            e8 = pp.tile([128, E8COLS], BF16_DT)
            wpre1 = pp.tile([128, 8 * 128], BF16_DT)
            wpre2 = pp.tile([128, 8 * 64], BF16_DT)
            bias1 = pp.tile([1, 128], BF16_DT)
            bias2 = pp.tile([1, 64], BF16_DT)
            gse = pp.tile([64, 128], BF16_DT)
            gso = pp.tile([64, 128], BF16_DT)
            wsing = pp.tile([64, 8 * 128], BF16_DT)
            wpair = pp.tile([128, 12 * 128], BF16_DT)
            wbT = pp.tile([64, 64], BF16_DT)
            bw65 = pp.tile([65, 64], BF16_DT)
            decstat = pp.tile([65, 256], BF16_DT)
            gaug = pp.tile([65, NB], BF16_DT)
            ones65 = pp.tile([65, 1], BF16_DT)
            ones128 = pp.tile([128, 1], BF16_DT)

            for sb, dr in [
                (e8, e8_d), (wpre1, wpre1_d), (wpre2, wpre2_d),
                (bias1, bias1_d), (bias2, bias2_d),
                (gse, gse_d), (gso, gso_d),
                (wsing, wsing_d), (wpair, wpair_d), (wbT, wbT_d),
                (bw65, bw65_d), (decstat, decstat_d),
                (gaug, gaug_d), (ones65, ones65_d), (ones128, ones128_d),
            ]:
                nc.default_dma_engine.dma_start(sb[:], dr[:])

            preg = pp.tile([128, N, 4], F32)      # pre1 (dup) | pre2 per t
            m_hist = pp.tile([65, NB + 2 * BL], BF16_DT)  # row 64 == 1.0
            m4 = pp.tile([128, 4], BF16_DT)
            h1d = pp.tile([128, BL], BF16_DT)
            h2t = pp.tile([65, BL], BF16_DT)      # row 64 == 1.0
            tsb = pp.tile([128, 32, BL], BF16_DT)
            onerow = pp.tile([1, 512], BF16_DT)
            zeros4 = pp.tile([64, 4], BF16_DT)

            nc.vector.memset(m_hist[0:64, 0:BL], 0.0)
            nc.vector.memset(m_hist[64:65, :], 1.0)
            nc.vector.memset(m4[:], 0.0)
            nc.vector.memset(h2t[64:65, :], 1.0)
            nc.vector.memset(onerow[:], 1.0)
            nc.vector.memset(zeros4[:], 0.0)

            # ---- precompute preg = [pre1(b0),pre1(b1),pre2(b0),pre2(b1)] ----
            for n in range(8):
                ps1 = psp.tile([128, 256, BL], F32, tag="big")
                for c in range(8):
                    nc.tensor.matmul(
                        ps1[:],
                        wpre1[:, c * 128:(c + 1) * 128],
                        e8[:, 16 * c + 512 * n: 16 * c + 512 * n + 512],
                        start=(c == 0), stop=False,
                    )
                nc.tensor.matmul(ps1[:], bias1[:], onerow[:],
                                 start=False, stop=True, skip_group_check=True)
                nc.vector.tensor_copy(preg[:, 256 * n:256 * (n + 1), 0:2],
                                      ps1[:])
                ps2 = psp.tile([64, 256, BL], F32, tag="big")
                for c in range(8):
                    nc.tensor.matmul(
                        ps2[:],
                        wpre2[:, c * 64:(c + 1) * 64],
                        e8[:, 16 * c + 512 * n: 16 * c + 512 * n + 512],
                        start=(c == 0), stop=False,
                    )
                nc.tensor.matmul(ps2[:], bias2[:], onerow[:],
                                 start=False, stop=True, skip_group_check=True)
                nc.vector.tensor_copy(preg[0:64, 256 * n:256 * (n + 1), 2:4],
                                      ps2[:])

            # warm up the two g PSUM banks so their has_written bits are set
            # (the in-loop g matmuls use start=False over a DVE pre-write)
            for _ in range(2):
                gw = psp.tile([128, 1, 4], F32, tag="g_ps")
                nc.tensor.matmul(gw[:, 0, :], gse[:], zeros4[:],
                                 start=True, stop=True)
            # dummy sigmoid: loads the act table on the preamble path so the
            # fixpoint pass hoists the per-iteration ACT_TABLE_LOAD (1283ns)
            # out of the scan loop
            sigscratch = pp.tile([1, 1], F32)
            nc.scalar.activation(sigscratch[:], zeros4[0:1, 0:1], AF.Sigmoid)

            # ---- the scan ----
            with tc.For_i(0, N, unroll,
                          hint_engines=(mybir.EngineType.PE,)) as iv:
                for k in range(unroll):
                    t = iv + k
                    tcol = t * BL
                    g_ps = psp.tile([128, 1, 4], F32, tag="g_ps")
                    T_psA = psp.tile([128, 16, BL], F32, tag="T_psA", bufs=1)
                    T_psB = psp.tile([128, 16, BL], F32, tag="T_psB", bufs=1)

                    # pre1/pre2 pre-written into the g bank (vector, early)
                    nc.vector.tensor_copy(g_ps[:], preg[:, bass.ds(t, 1), :])

                    # g = [A1|A1]^T m (+pre1), [B1|B1]^T m (+pre2)
                    nc.tensor.matmul(g_ps[:, 0, 0:2], gse[:], m4[0:64, 0:2],
                                     start=False, stop=True,
                                     skip_group_check=True)
                    nc.tensor.matmul(g_ps[:, 0, 2:4], gso[:], m4[0:64, 0:2],
                                     start=False, stop=True,
                                     skip_group_check=True)
                    # T chunks: T[p, c, b]: p<64 -> T[h=p, i=c],
                    #                       p>=64 -> T[h=p-64, i=c+32]
                    # chunks 0-7: single [64,128] stationaries, need only the
                    # lower half of m4 (run while sig#2 fills the upper half)
                    for c in range(8):
                        nc.tensor.matmul(
                            T_psA[:, c, :],
                            wsing[:, c * 128:(c + 1) * 128],
                            m4[0:64, 0:2], start=True, stop=True)
                    # upper half of m4 for the pair chunks: partition-shift
                    # copy of the lower half on the scalar engine; emitted
                    # after the singles so they do not wait on it
                    nc.scalar.activation(m4[64:128, 2:4], m4[0:64, 0:2],
                                         AF.Copy)
                    # chunks 8-31: [128,128] pair stationaries, 4-col moving
                    # [m;0 | 0;m] computes two chunks per LDWEIGHTS
                    for p in range(4):
                        nc.tensor.matmul(
                            T_psA[:, 8 + 2 * p: 10 + 2 * p, :],
                            wpair[:, p * 128:(p + 1) * 128],
                            m4[:], start=True, stop=True)
                    for p in range(4, 12):
                        nc.tensor.matmul(
                            T_psB[:, 2 * p - 8: 2 * p - 6, :],
                            wpair[:, p * 128:(p + 1) * 128],
                            m4[:], start=True, stop=True)

                    # h = sigmoid(g) (g already includes pre via pre-write)
                    nc.scalar.activation(h1d[:], g_ps[:, 0, 0:2], AF.Sigmoid)
                    nc.scalar.activation(h2t[0:64, :], g_ps[0:64, 0, 2:4],
                                         AF.Sigmoid)

                    # T -> SBUF (bf16); A copy overlaps the B-chunk matmuls
                    nc.vector.tensor_copy(tsb[:, 0:16, :], T_psA[:])
                    nc.vector.tensor_copy(tsb[:, 16:32, :], T_psB[:])

                    # a = WbT@m + bw65@[h2;1] + sum_h h1*T
                    a_ps = psp.tile([64, BL], F32, tag="a_ps", bufs=1)
                    nc.tensor.matmul(a_ps[:], wbT[:], m4[0:64, 0:2],
                                     start=True, stop=False)
                    nc.tensor.matmul(a_ps[:], bw65[:], h2t[:],
                                     start=False, stop=False,
                                     skip_group_check=True)
                    for b in range(BL):
                        nc.tensor.matmul(a_ps[0:32, b: b + 1],
                                         tsb[0:64, :, b], h1d[0:64, b: b + 1],
                                         start=False, stop=False,
                                         skip_group_check=True,
                                         tile_position=(0, 0))
                        last = b == BL - 1
                        nc.tensor.matmul(a_ps[32:64, b: b + 1],
                                         tsb[64:128, :, b],
                                         h1d[64:128, b: b + 1],
                                         start=False, stop=last,
                                         skip_group_check=True,
                                         tile_position=(64, 32))

                    # m' = sigmoid(a), lower half; the upper half for the
                    # next step's pairs is produced early next iteration
                    nc.scalar.activation(m4[0:64, 0:2], a_ps[:], AF.Sigmoid)
                    nc.vector.tensor_copy(
                        m_hist[0:64, bass.ds(tcol + BL, BL)], m4[0:64, 0:2])

            # ---- bulk loss ----
            loss = pp.tile([1, NB], F32)
            with tc.tile_pool(name="bulk", bufs=2) as bp:
                for tcn in range(8):
                    sl = slice(512 * tcn, 512 * (tcn + 1))
                    msl = slice(BL + 512 * tcn, BL + 512 * (tcn + 1))
                    se_ps = psp.tile([1, 512], F32, tag="seps", bufs=1)
                    for half in range(2):
                        lg_ps = psp.tile([128, 512], F32, tag="big")
                        exps = bp.tile([128, 512], BF16_DT, tag="exps")
                        nc.tensor.matmul(
                            lg_ps[:],
                            decstat[:, half * 128:(half + 1) * 128],
                            m_hist[:, msl],
                            start=True, stop=True)
                        nc.scalar.activation(exps[:], lg_ps[:], AF.Exp)
                        nc.tensor.matmul(se_ps[:], ones128[:], exps[:],
                                         start=(half == 0), stop=(half == 1))
                    lse_t = bp.tile([1, 512], F32, tag="lse")
                    nc.scalar.activation(lse_t[:], se_ps[:], AF.Ln)
                    paug_t = bp.tile([65, 512], BF16_DT, tag="paug")
                    nc.vector.tensor_tensor(paug_t[:], gaug[:, sl],
                                            m_hist[:, msl],
                                            mybir.AluOpType.mult)
                    pk_ps = psp.tile([1, 512], F32, tag="T_psA", bufs=1)
                    nc.tensor.matmul(pk_ps[:], ones65[:], paug_t[:],
                                     start=True, stop=True)
                    # loss = lse*inv_ln2 - pick (pick already scaled on host)
                    nc.vector.scalar_tensor_tensor(
                        loss[:, sl], lse_t[:], INV_LN2, pk_ps[:],
                        mybir.AluOpType.mult, mybir.AluOpType.subtract)
            nc.default_dma_engine.dma_start(out_d[:], loss[:])

    nc.compile()
    return nc


def _prep_core_inputs(x0, emb, W_enc_w, W_enc_b, W_dec_w, W_dec_b,
                      b_enc_w, b_enc_b, b_dec_w, b_dec_b, dec_w, dec_b):
    """Host-side gathers/packing -> list of per-core input dicts."""
    f32 = np.float32
    x0 = np.asarray(x0)
    xp = np.concatenate([np.zeros((B, L), x0.dtype), x0], axis=1)  # [B, N+L]
    e = np.asarray(emb, f32)[xp]  # [B, N+L, E]

    # shared weight packs
    Wcat = np.concatenate([np.asarray(W_enc_w, f32), np.asarray(b_enc_w, f32)],
                          axis=1)  # [1088, 128]
    wpre1 = np.zeros((128, 8 * 128), f32)
    wpre2 = np.zeros((128, 8 * 64), f32)
    for c in range(8):
        blk = Wcat[64 + 128 * c: 64 + 128 * (c + 1)]  # [128, 128]
        wpre1[:, c * 128: c * 128 + 64] = blk[:, :64]
        wpre1[:, c * 128 + 64: c * 128 + 128] = blk[:, :64]
        wpre2[:, c * 64:(c + 1) * 64] = blk[:, 64:]
    bias1 = np.concatenate([np.asarray(W_enc_b, f32)] * 2).reshape(1, 128)
    bias2 = np.asarray(b_enc_b, f32).reshape(1, 64)
    gse = np.concatenate([Wcat[:64, :64]] * 2, axis=1)    # [64, 128]
    gso = np.concatenate([Wcat[:64, 64:128]] * 2, axis=1)  # [64, 128]

    W2r = np.asarray(W_dec_w, f32).reshape(H, M, M)  # [h, i, j]
    wstatT = np.zeros((64, 32 * 128), f32)
    for c in range(32):
        wstatT[:, c * 128: c * 128 + 64] = W2r[:, c, :].T
        wstatT[:, c * 128 + 64: c * 128 + 128] = W2r[:, c + 32, :].T
    wbT = np.asarray(W_dec_b, f32).reshape(M, M).T.copy()  # [j, i]
    bw65 = np.concatenate([np.asarray(b_dec_w, f32),
                           np.asarray(b_dec_b, f32).reshape(1, 64)], axis=0)
    decstat = np.concatenate([np.asarray(dec_w, f32),
                              np.asarray(dec_b, f32).reshape(1, 256)], axis=0)
    ones65 = np.ones((65, 1), f32)
    ones128 = np.ones((128, 1), f32)

    wsing = wstatT[:, 0: 8 * 128]
    wpair = np.zeros((128, 12 * 128), f32)
    for p in range(12):
        wpair[0:64, p * 128:(p + 1) * 128] = \
            wstatT[:, (8 + 2 * p) * 128:(9 + 2 * p) * 128]
        wpair[64:128, p * 128:(p + 1) * 128] = \
            wstatT[:, (9 + 2 * p) * 128:(10 + 2 * p) * 128]
    shared = dict(
        wpre1=wpre1.astype(BF16), wpre2=wpre2.astype(BF16),
        bias1=bias1.astype(BF16), bias2=bias2.astype(BF16),
        gse=gse.astype(BF16), gso=gso.astype(BF16),
        wsing=wsing.astype(BF16), wpair=wpair.astype(BF16),
        wbT=wbT.astype(BF16),
        bw65=bw65.astype(BF16),
        decstat=decstat.astype(BF16),
        ones65=ones65.astype(BF16), ones128=ones128.astype(BF16),
    )

    in_maps = []
    inv_ln2 = np.float32(1.0 / np.log(2.0))
    dec_wT = np.asarray(dec_w, f32).T.copy()  # [256, 64]
    dec_bv = np.asarray(dec_b, f32)
    for k in range(NCORES):
        rows = slice(BL * k, BL * (k + 1))
        ek = e[rows]  # [BL, N+L, E]
        # e8[l_sub*16+eps, tau*BL+b] = ek[b, tau+l_sub, eps]
        e8 = np.zeros((128, E8COLS), f32)
        for ls in range(8):
            blk = ek[:, ls: ls + TAU, :].transpose(2, 1, 0)  # [E, TAU, BL]
            e8[ls * 16:(ls + 1) * 16] = blk.reshape(E, E8COLS)
        y = np.asarray(x0[rows])  # [BL, N]
        g = dec_wT[y]  # [BL, N, 64]
        gaug = np.zeros((65, NB), f32)
        gaug[:64] = g.transpose(2, 1, 0).reshape(64, NB)
        gaug[64] = dec_bv[y].T.reshape(NB)
        gaug *= inv_ln2
        d = dict(shared)
        d["e8"] = e8.astype(BF16)
        d["gaug"] = gaug.astype(BF16)
        in_maps.append(d)
    return in_maps


def kernel(**inputs):
    key = "nc"
    if key not in _cache:
        _cache[key] = _build_nc()
    nc = _cache[key]
    in_maps = _prep_core_inputs(**inputs)
    res = run_bass_kernel_spmd(nc, in_maps, list(range(NCORES)),
                               trace=bool(os.environ.get("KERNEL_TRACE")))
    _cache["last_result"] = res
    out = np.zeros((N, B), np.float32)
    for k in range(NCORES):
        out[:, BL * k: BL * (k + 1)] = res.results[k]["out"].reshape(N, BL)
    return out.reshape(-1)


# revision 27
# speedup vs baseline: 1.2496x; 1.0203x over previous
"""Trainium2 Bass kernel for nn_Net4 (hypernetwork RNN scan), v2.

Model (per step t, per batch row b):
  h1 = sigmoid(m @ A1 + pre1[t])          A1 = W_enc_w[:64]
  h2 = sigmoid(m @ B1 + pre2[t])          B1 = b_enc_w[:64]
  Wm = (h1 @ W_dec_w + W_dec_b).reshape(64,64)
  bm = h2 @ b_dec_w + b_dec_b
  m' = sigmoid(Wm @ m + bm)
  loss[t] = (logsumexp(m'@dec_w+dec_b) - (m'@dec_w+dec_b)[y]) / ln2

Final design (7.24ms baseline -> 4.10ms, rel err 4.0e-4):
  - everything in the scan is bf16 (v1 had fp32 wbT/bw65/tsb matmuls that
    lower to 2x LDWEIGHTS+MATMUL LOW/HIGH pairs at ~200ns each)
  - pre1/pre2 are pre-written into the g PSUM bank by the vector engine
    (interleaved preg layout), and the g matmuls accumulate on top with
    start=False (after a PE warmup that sets the has_written bits); this
    removes the vector add from the critical path
  - m4 [128,4] = [m;0 | 0;m]: only one sigmoid (lower half) on the step
    boundary; the upper half is a partition-shift Act copy emitted AFTER
    the single-chunk matmuls of the next step so they never wait on it
  - T block: 8 single [64,128] chunk stationaries (lower-half m only,
    run during the upper-half copy) + 12 paired [128,128] stationaries
    whose 4-col moving [m;0|0;m] yields two chunks per LDWEIGHTS
    (zero blocks kill the cross terms); more pairs is a net loss since
    a [128,128] LDWEIGHTS costs ~2x a [64,128] one
  - T PSUM split into two tiles (24+8 slots) so the big copy to SBUF
    overlaps the tail of the T matmuls and only a small 8-slot copy sits
    on the critical path (Tile dependency tracking is per-tile)
  - a dummy sigmoid before the loop keeps the act-table load out of the
    loop body (it cost 1283ns per For_i iteration otherwise)
  - m_hist is a bf16 copy of m done by the vector engine (off the
    critical path), feeding a bf16 bulk loss phase (v1 bulk was fp32 and
    took ~106us)
  - For_i unroll=256; each loop boundary costs ~5us (all-engine
    barrier + semaphore resets), so fewer boundaries is a direct win

Sharding: batch rows 2k,2k+1 -> core k; zero cross-core communication.
"""

import os
import sys
import numpy as np

sys.path.insert(0, "/opt/trn_rl_repo")

import concourse.bass as bass
import concourse.bacc as bacc
import concourse.mybir as mybir
import concourse.tile as tile
from concourse.bass_utils import run_bass_kernel_spmd

import ml_dtypes

BF16 = ml_dtypes.bfloat16

Cin, E, L, M, H, Cout = 256, 16, 64, 64, 64, 256
B, N = 16, 2048
D = M + L * E  # 1088
NCORES = 8
BL = B // NCORES  # 2 batch rows per core
NB = N * BL       # 4096 (t,b) pairs per core
TAU = N + L - 8   # e8 time length: tau in [0, 2104)
E8COLS = TAU * BL  # 4208

F32 = mybir.dt.float32
BF16_DT = mybir.dt.bfloat16
AF = mybir.ActivationFunctionType
FP8_DT = mybir.dt.float8e4
FP8 = ml_dtypes.float8_e4m3fn
INV_LN2 = float(1.0 / np.log(2.0))

_cache = {}


def _build_nc(unroll=256):
    nc = bacc.Bacc("TRN2", target_bir_lowering=False, debug=True)

    # ---- DRAM parameters (per-core inputs) ----
    def P(name, shape, dt):
        return nc.declare_dram_parameter(name, list(shape), dt, isOutput=False)

    e8_d = P("e8", (128, E8COLS), BF16_DT)
    wpre1_d = P("wpre1", (128, 8 * 128), BF16_DT)
    wpre2_d = P("wpre2", (128, 8 * 64), BF16_DT)
    bias1_d = P("bias1", (1, 128), BF16_DT)   # [W_enc_b | W_enc_b]
    bias2_d = P("bias2", (1, 64), BF16_DT)    # b_enc_b
    gse_d = P("gse", (64, 128), BF16_DT)      # [A1 | A1]
    gso_d = P("gso", (64, 128), BF16_DT)      # [B1 | B1]
    wsing_d = P("wsing", (64, 8 * 128), BF16_DT)
    wpair_d = P("wpair", (128, 12 * 128), BF16_DT)
    wbT_d = P("wbT", (64, 64), BF16_DT)       # W_dec_b reshaped [j,i]
    bw65_d = P("bw65", (65, 64), BF16_DT)     # [b_dec_w ; b_dec_b]
    decstat_d = P("decstat", (65, 256), BF16_DT)  # [dec_w ; dec_b]
    gaug_d = P("gaug", (65, NB), BF16_DT)     # picked dec cols * inv_ln2
    ones65_d = P("ones65", (65, 1), BF16_DT)
    ones128_d = P("ones128", (128, 1), BF16_DT)
    out_d = nc.declare_dram_parameter("out", [1, NB], F32, isOutput=True)

    with tile.TileContext(nc) as tc:
        with (
            tc.tile_pool(name="persist", bufs=1) as pp,
            tc.tile_pool(name="psum", bufs=2, space="PSUM") as psp,
        ):
            e8 = pp.tile([128, E8COLS], BF16_DT)
            wpre1 = pp.tile([128, 8 * 128], BF16_DT)
            wpre2 = pp.tile([128, 8 * 64], BF16_DT)
            bias1 = pp.tile([1, 128], BF16_DT)
            bias2 = pp.tile([1, 64], BF16_DT)
            gse = pp.tile([64, 128], BF16_DT)
            gso = pp.tile([64, 128], BF16_DT)
            wsing = pp.tile([64, 8 * 128], BF16_DT)
            wpair = pp.tile([128, 12 * 128], BF16_DT)
            wbT = pp.tile([64, 64], BF16_DT)
            bw65 = pp.tile([65, 64], BF16_DT)
            decstat = pp.tile([65, 256], BF16_DT)
            gaug = pp.tile([65, NB], BF16_DT)
            ones65 = pp.tile([65, 1], BF16_DT)
            ones128 = pp.tile([128, 1], BF16_DT)

            for sb, dr in [
                (e8, e8_d), (wpre1, wpre1_d), (wpre2, wpre2_d),
                (bias1, bias1_d), (bias2, bias2_d),
                (gse, gse_d), (gso, gso_d),
                (wsing, wsing_d), (wpair, wpair_d), (wbT, wbT_d),
                (bw65, bw65_d), (decstat, decstat_d),
                (gaug, gaug_d), (ones65, ones65_d), (ones128, ones128_d),
            ]:
                nc.default_dma_engine.dma_start(sb[:], dr[:])

            preg = pp.tile([128, N, 4], F32)      # pre1 (dup) | pre2 per t
            m_hist = pp.tile([65, NB + 2 * BL], BF16_DT)  # row 64 == 1.0
            m4 = pp.tile([128, 4], BF16_DT)
            h1d = pp.tile([128, BL], BF16_DT)
            h2t = pp.tile([65, BL], BF16_DT)      # row 64 == 1.0
            tsb = pp.tile([128, 32, BL], BF16_DT)
            onerow = pp.tile([1, 512], BF16_DT)
            zeros4 = pp.tile([64, 4], BF16_DT)

            nc.vector.memset(m_hist[0:64, 0:BL], 0.0)
            nc.vector.memset(m_hist[64:65, :], 1.0)
            nc.vector.memset(m4[:], 0.0)
            nc.vector.memset(h2t[64:65, :], 1.0)
            nc.vector.memset(onerow[:], 1.0)
            nc.vector.memset(zeros4[:], 0.0)

            # ---- precompute preg = [pre1(b0),pre1(b1),pre2(b0),pre2(b1)] ----
            for n in range(8):
                ps1 = psp.tile([128, 256, BL], F32, tag="big")
                for c in range(8):
                    nc.tensor.matmul(
                        ps1[:],
                        wpre1[:, c * 128:(c + 1) * 128],
                        e8[:, 16 * c + 512 * n: 16 * c + 512 * n + 512],
                        start=(c == 0), stop=False,
                    )
                nc.tensor.matmul(ps1[:], bias1[:], onerow[:],
                                 start=False, stop=True, skip_group_check=True)
                nc.vector.tensor_copy(preg[:, 256 * n:256 * (n + 1), 0:2],
                                      ps1[:])
                ps2 = psp.tile([64, 256, BL], F32, tag="big")
                for c in range(8):
                    nc.tensor.matmul(
                        ps2[:],
                        wpre2[:, c * 64:(c + 1) * 64],
                        e8[:, 16 * c + 512 * n: 16 * c + 512 * n + 512],
                        start=(c == 0), stop=False,
                    )
                nc.tensor.matmul(ps2[:], bias2[:], onerow[:],
                                 start=False, stop=True, skip_group_check=True)
                nc.vector.tensor_copy(preg[0:64, 256 * n:256 * (n + 1), 2:4],
                                      ps2[:])

            # warm up the two g PSUM banks so their has_written bits are set
            # (the in-loop g matmuls use start=False over a DVE pre-write)
            for _ in range(2):
                gw = psp.tile([128, 1, 4], F32, tag="g_ps")
                nc.tensor.matmul(gw[:, 0, :], gse[:], zeros4[:],
                                 start=True, stop=True)
            # dummy sigmoid: loads the act table on the preamble path so the
            # fixpoint pass hoists the per-iteration ACT_TABLE_LOAD (1283ns)
            # out of the scan loop
            sigscratch = pp.tile([1, 1], F32)
            nc.scalar.activation(sigscratch[:], zeros4[0:1, 0:1], AF.Sigmoid)

            # ---- the scan ----
            with tc.For_i(0, N, unroll,
                          hint_engines=(mybir.EngineType.PE,)) as iv:
                for k in range(unroll):
                    t = iv + k
                    tcol = t * BL
                    g_ps = psp.tile([128, 1, 4], F32, tag="g_ps")
                    T_psA = psp.tile([128, 24, BL], F32, tag="T_psA", bufs=1)
                    T_psB = psp.tile([128, 8, BL], F32, tag="T_psB", bufs=1)

                    # pre1/pre2 pre-written into the g bank (vector, early)
                    nc.vector.tensor_copy(g_ps[:], preg[:, bass.ds(t, 1), :])

                    # g = [A1|A1]^T m (+pre1), [B1|B1]^T m (+pre2)
                    nc.tensor.matmul(g_ps[:, 0, 0:2], gse[:], m4[0:64, 0:2],
                                     start=False, stop=True,
                                     skip_group_check=True)
                    nc.tensor.matmul(g_ps[:, 0, 2:4], gso[:], m4[0:64, 0:2],
                                     start=False, stop=True,
                                     skip_group_check=True)
                    # T chunks: T[p, c, b]: p<64 -> T[h=p, i=c],
                    #                       p>=64 -> T[h=p-64, i=c+32]
                    # chunks 0-7: single [64,128] stationaries, need only the
                    # lower half of m4 (run while sig#2 fills the upper half)
                    for c in range(8):
                        nc.tensor.matmul(
                            T_psA[:, c, :],
                            wsing[:, c * 128:(c + 1) * 128],
                            m4[0:64, 0:2], start=True, stop=True)
                    # upper half of m4 for the pair chunks: partition-shift
                    # copy of the lower half on the scalar engine; emitted
                    # after the singles so they do not wait on it
                    nc.scalar.activation(m4[64:128, 2:4], m4[0:64, 0:2],
                                         AF.Copy)
                    # chunks 8-31: [128,128] pair stationaries, 4-col moving
                    # [m;0 | 0;m] computes two chunks per LDWEIGHTS
                    for p in range(8):
                        nc.tensor.matmul(
                            T_psA[:, 8 + 2 * p: 10 + 2 * p, :],
                            wpair[:, p * 128:(p + 1) * 128],
                            m4[:], start=True, stop=True)
                    for p in range(8, 12):
                        nc.tensor.matmul(
                            T_psB[:, 2 * p - 16: 2 * p - 14, :],
                            wpair[:, p * 128:(p + 1) * 128],
                            m4[:], start=True, stop=True)

                    # h = sigmoid(g) (g already includes pre via pre-write)
                    nc.scalar.activation(h1d[:], g_ps[:, 0, 0:2], AF.Sigmoid)
                    nc.scalar.activation(h2t[0:64, :], g_ps[0:64, 0, 2:4],
                                         AF.Sigmoid)

                    # T -> SBUF (bf16); A copy overlaps the B-chunk matmuls
                    nc.vector.tensor_copy(tsb[:, 0:24, :], T_psA[:])
                    nc.vector.tensor_copy(tsb[:, 24:32, :], T_psB[:])

                    # a = WbT@m + bw65@[h2;1] + sum_h h1*T
                    a_ps = psp.tile([64, BL], F32, tag="a_ps", bufs=1)
                    nc.tensor.matmul(a_ps[:], wbT[:], m4[0:64, 0:2],
                                     start=True, stop=False)
                    nc.tensor.matmul(a_ps[:], bw65[:], h2t[:],
                                     start=False, stop=False,
                                     skip_group_check=True)
                    for b in range(BL):
                        nc.tensor.matmul(a_ps[0:32, b: b + 1],
                                         tsb[0:64, :, b], h1d[0:64, b: b + 1],
                                         start=False, stop=False,
                                         skip_group_check=True,
                                         tile_position=(0, 0))
                        last = b == BL - 1
                        nc.tensor.matmul(a_ps[32:64, b: b + 1],
                                         tsb[64:128, :, b],
                                         h1d[64:128, b: b + 1],
                                         start=False, stop=last,
                                         skip_group_check=True,
                                         tile_position=(64, 32))

                    # m' = sigmoid(a), lower half; the upper half for the
                    # next step's pairs is produced early next iteration
                    nc.scalar.activation(m4[0:64, 0:2], a_ps[:], AF.Sigmoid)
                    nc.vector.tensor_copy(
                        m_hist[0:64, bass.ds(tcol + BL, BL)], m4[0:64, 0:2])

            # ---- bulk loss ----
            loss = pp.tile([1, NB], F32)
            with tc.tile_pool(name="bulk", bufs=2) as bp:
                for tcn in range(8):
                    sl = slice(512 * tcn, 512 * (tcn + 1))
                    msl = slice(BL + 512 * tcn, BL + 512 * (tcn + 1))
                    se_ps = psp.tile([1, 512], F32, tag="seps", bufs=1)
                    for half in range(2):
                        lg_ps = psp.tile([128, 512], F32, tag="big")
                        exps = bp.tile([128, 512], BF16_DT, tag="exps")
                        nc.tensor.matmul(
                            lg_ps[:],
                            decstat[:, half * 128:(half + 1) * 128],
                            m_hist[:, msl],
                            start=True, stop=True)
                        nc.scalar.activation(exps[:], lg_ps[:], AF.Exp)
                        nc.tensor.matmul(se_ps[:], ones128[:], exps[:],
                                         start=(half == 0), stop=(half == 1))
                    lse_t = bp.tile([1, 512], F32, tag="lse")
                    nc.scalar.activation(lse_t[:], se_ps[:], AF.Ln)
                    paug_t = bp.tile([65, 512], BF16_DT, tag="paug")
                    nc.vector.tensor_tensor(paug_t[:], gaug[:, sl],
                                            m_hist[:, msl],
                                            mybir.AluOpType.mult)
                    pk_ps = psp.tile([1, 512], F32, tag="T_psA", bufs=1)
                    nc.tensor.matmul(pk_ps[:], ones65[:], paug_t[:],
                                     start=True, stop=True)
                    # loss = lse*inv_ln2 - pick (pick already scaled on host)
                    nc.vector.scalar_tensor_tensor(
                        loss[:, sl], lse_t[:], INV_LN2, pk_ps[:],
                        mybir.AluOpType.mult, mybir.AluOpType.subtract)
            nc.default_dma_engine.dma_start(out_d[:], loss[:])

    nc.compile()
    return nc


def _prep_core_inputs(x0, emb, W_enc_w, W_enc_b, W_dec_w, W_dec_b,
                      b_enc_w, b_enc_b, b_dec_w, b_dec_b, dec_w, dec_b):
    """Host-side gathers/packing -> list of per-core input dicts."""
    f32 = np.float32
    x0 = np.asarray(x0)
    xp = np.concatenate([np.zeros((B, L), x0.dtype), x0], axis=1)  # [B, N+L]
    e = np.asarray(emb, f32)[xp]  # [B, N+L, E]

    # shared weight packs
    Wcat = np.concatenate([np.asarray(W_enc_w, f32), np.asarray(b_enc_w, f32)],
                          axis=1)  # [1088, 128]
    wpre1 = np.zeros((128, 8 * 128), f32)
    wpre2 = np.zeros((128, 8 * 64), f32)
    for c in range(8):
        blk = Wcat[64 + 128 * c: 64 + 128 * (c + 1)]  # [128, 128]
        wpre1[:, c * 128: c * 128 + 64] = blk[:, :64]
        wpre1[:, c * 128 + 64: c * 128 + 128] = blk[:, :64]
        wpre2[:, c * 64:(c + 1) * 64] = blk[:, 64:]
    bias1 = np.concatenate([np.asarray(W_enc_b, f32)] * 2).reshape(1, 128)
    bias2 = np.asarray(b_enc_b, f32).reshape(1, 64)
    gse = np.concatenate([Wcat[:64, :64]] * 2, axis=1)    # [64, 128]
    gso = np.concatenate([Wcat[:64, 64:128]] * 2, axis=1)  # [64, 128]

    W2r = np.asarray(W_dec_w, f32).reshape(H, M, M)  # [h, i, j]
    wstatT = np.zeros((64, 32 * 128), f32)
    for c in range(32):
        wstatT[:, c * 128: c * 128 + 64] = W2r[:, c, :].T
        wstatT[:, c * 128 + 64: c * 128 + 128] = W2r[:, c + 32, :].T
    wbT = np.asarray(W_dec_b, f32).reshape(M, M).T.copy()  # [j, i]
    bw65 = np.concatenate([np.asarray(b_dec_w, f32),
                           np.asarray(b_dec_b, f32).reshape(1, 64)], axis=0)
    decstat = np.concatenate([np.asarray(dec_w, f32),
                              np.asarray(dec_b, f32).reshape(1, 256)], axis=0)
    ones65 = np.ones((65, 1), f32)
    ones128 = np.ones((128, 1), f32)

    wsing = wstatT[:, 0: 8 * 128]
    wpair = np.zeros((128, 12 * 128), f32)
    for p in range(12):
        wpair[0:64, p * 128:(p + 1) * 128] = \
            wstatT[:, (8 + 2 * p) * 128:(9 + 2 * p) * 128]
        wpair[64:128, p * 128:(p + 1) * 128] = \
            wstatT[:, (9 + 2 * p) * 128:(10 + 2 * p) * 128]
    shared = dict(
        wpre1=wpre1.astype(BF16), wpre2=wpre2.astype(BF16),
        bias1=bias1.astype(BF16), bias2=bias2.astype(BF16),
        gse=gse.astype(BF16), gso=gso.astype(BF16),
        wsing=wsing.astype(BF16), wpair=wpair.astype(BF16),
        wbT=wbT.astype(BF16),
        bw65=bw65.astype(BF16),
        decstat=decstat.astype(BF16),
        ones65=ones65.astype(BF16), ones128=ones128.astype(BF16),
    )

    in_maps = []
    inv_ln2 = np.float32(1.0 / np.log(2.0))
    dec_wT = np.asarray(dec_w, f32).T.copy()  # [256, 64]
    dec_bv = np.asarray(dec_b, f32)
    for k in range(NCORES):
        rows = slice(BL * k, BL * (k + 1))
        ek = e[rows]  # [BL, N+L, E]
        # e8[l_sub*16+eps, tau*BL+b] = ek[b, tau+l_sub, eps]
        e8 = np.zeros((128, E8COLS), f32)
        for ls in range(8):
            blk = ek[:, ls: ls + TAU, :].transpose(2, 1, 0)  # [E, TAU, BL]
            e8[ls * 16:(ls + 1) * 16] = blk.reshape(E, E8COLS)
        y = np.asarray(x0[rows])  # [BL, N]
        g = dec_wT[y]  # [BL, N, 64]
        gaug = np.zeros((65, NB), f32)
        gaug[:64] = g.transpose(2, 1, 0).reshape(64, NB)
        gaug[64] = dec_bv[y].T.reshape(NB)
        gaug *= inv_ln2
        d = dict(shared)
        d["e8"] = e8.astype(BF16)
        d["gaug"] = gaug.astype(BF16)
        in_maps.append(d)
    return in_maps


def kernel(**inputs):
    key = "nc"
    if key not in _cache:
        _cache[key] = _build_nc()
    nc = _cache[key]
    in_maps = _prep_core_inputs(**inputs)
    res = run_bass_kernel_spmd(nc, in_maps, list(range(NCORES)),
                               trace=bool(os.environ.get("KERNEL_TRACE")))
    _cache["last_result"] = res
    out = np.zeros((N, B), np.float32)
    for k in range(NCORES):
        out[:, BL * k: BL * (k + 1)] = res.results[k]["out"].reshape(N, BL)
    return out.reshape(-1)
